# revision 12
# baseline (speedup 1.0000x reference)
"""GCNII (64-layer) + MLP head on 8 Trainium2 NeuronCores.

Strategy (node-sharded graph parallel, v2):
  - Nodes sharded contiguously across 8 cores (12500 each). Each core owns the
    segment-sum for its destination nodes.
  - The replicated node-feature table holds dinv-scaled x in bf16, PAIR-packed:
    row k of xfullA/xfullB = nodes (2k, 2k+1) -> 128 bf16 = 256B rows, the
    minimum dma_gather elem size. Two tables: A = each core's node rows
    0..6143 (windows 0-11), B = rows 6144..12499 (windows 12-24), so the
    per-layer AllGather splits into AG-A / AG-B that overlap compute.
  - Self-loops are NOT gathered; they fold into the per-window update as
    0.9*dinv^2*x_old (DVE), which also removes cross-core chunk padding skew.
  - Per layer, two phases over 13 dst-window blocks (2 windows each):
      phase A: gather bucket-A rows (edge srcs with src-offset < 6144),
        indicator matmuls (one per parity: even/odd node of the pair) into
        psum, copy psum -> aggA rows of T1.
      phase B: gather bucket-B rows, matmuls into psum, then per window
        h = 0.9*dinv*(aggA + pswB) + 0.1*x0 + 0.9*dinv^2*x_old,
        x = relu(h @ W'l) with W'l = (1-b)I + b*Wl folded on the host,
        transpose to pair-packed bf16 slabs; AG-A fires mid-phase-B
        (after window 11), AG-B at layer end.
  - PSUM column offsets are per-core data (loaded into PE registers from SBUF)
    so a single SPMD program serves all 8 cores; chunk-count templates are
    cross-core maxima.
"""
import os
import numpy as np
import ml_dtypes

# problem dims
N, F, H, L, R, C, E = 100000, 500, 64, 64, 512, 40, 1000000
ALPHA, THETA = 0.1, 0.5
M1, M2 = (R - H) // 3 + H, 2 * ((R - H) // 3) + H  # 213, 362
NCORES = 8
NP = N // NCORES          # 12500
WIN = 512                 # dst window width
NWIN = (NP + WIN - 1) // WIN   # 25 (last window 212 wide)
WBLOCK = 1                # windows per gather-call block
NBLK = (NWIN + WBLOCK - 1) // WBLOCK  # 25
NQUEUES = 4               # SWDGE queues (gathers on distinct queues overlap)
AH = 6144                 # nodes per core in half A (windows 0-11)
BH = NP - AH              # 6356 (windows 12-24)
APAIR, BPAIR = AH // 2, BH // 2  # 3072, 3178
AROWS, BROWS = NCORES * APAIR, NCORES * BPAIR  # 24576, 25424 (< 32768)
INDW = 96                 # indicator width
NLAYERS = int(os.environ.get("GCN_NLAYERS", str(L)))
AGA_BLK = 11              # phase-B block after which windows 0-11 are done

F32 = np.float32
BF16 = ml_dtypes.bfloat16


def _winw(w):
    return min(WIN, NP - w * WIN)


def _nbins(w):
    return (_winw(w) + INDW - 1) // INDW


def _boff(w, b):
    """Static PSUM column offset of bin b in window w (compile-time)."""
    return min(b * INDW, max(0, _winw(w) - INDW))


NB = _nbins(0)  # bins per full window (6)


def preprocess(edge_index):
    """Host-side graph preprocessing. Returns (meta, per-core host data).

    Edges are grouped by (dst window, src half, 96-wide dst-position bin);
    each bin's chunks share a compile-time PSUM column offset, so the device
    needs no register-loaded offsets. Every bin gets >= 1 template chunk so
    the matmul accumulation group writes every PSUM column of the window
    (untouched columns would keep stale bank data)."""
    src = edge_index[0].astype(np.int64)
    dst = edge_index[1].astype(np.int64)
    deg = (np.bincount(dst, minlength=N) + 1.0).astype(F32)  # +1 self loop
    dinv = (1.0 / np.sqrt(deg)).astype(F32)

    cores = []
    counts = np.zeros((NCORES, NWIN, 2, NB), np.int64)
    for i in range(NCORES):
        m = (dst // NP) == i
        s = src[m]
        d = dst[m] - i * NP
        w = d // WIN
        dloc = d - w * WIN
        soff = s % NP
        half = (soff >= AH).astype(np.int64)
        bn = dloc // INDW
        order = np.lexsort((dloc, bn, half, w))
        s, d, half, bn = s[order], d[order], half[order], bn[order]
        w = w[order]
        key = (w * 2 + half) * NB + bn
        cnt = np.bincount(key, minlength=NWIN * 2 * NB).reshape(NWIN, 2, NB)
        counts[i] = cnt
        cores.append((s, d, cnt))

    T = ((counts + 127) // 128).max(axis=0)  # [NWIN, 2, NB]
    for w in range(NWIN):
        nb = _nbins(w)
        T[w, :, :nb] = np.maximum(T[w, :, :nb], 1)  # PSUM coverage
        assert (T[w, :, nb:] == 0).all()

    # stream order: phase (A=0/B=1) -> block -> window -> bins -> chunks
    chunk_base = np.zeros((NWIN, 2), np.int64)
    chunk_off = [[None] * 2 for _ in range(NWIN)]
    call_e0 = np.zeros((2, NBLK), np.int64)
    call_edges = np.zeros((2, NBLK), np.int64)
    cb = 0
    for ph in range(2):
        for b in range(NBLK):
            call_e0[ph][b] = cb * 128
            for w in range(b * WBLOCK, min((b + 1) * WBLOCK, NWIN)):
                chunk_base[w][ph] = cb
                offs = []
                for bn in range(_nbins(w)):
                    offs += [_boff(w, bn)] * int(T[w][ph][bn])
                chunk_off[w][ph] = tuple(offs)
                cb += len(offs)
            call_edges[ph][b] = cb * 128 - call_e0[ph][b]
    nch = cb
    ntot = nch * 128

    per_core = []
    for i in range(NCORES):
        s, d, cnt = cores[i]
        idxs = np.zeros(ntot, np.int16)
        segsA = np.full(ntot, -1.0, F32)
        segsB = np.full(ntot, -1.0, F32)
        gstart = np.zeros(NWIN * 2 * NB + 1, np.int64)
        np.cumsum(cnt.reshape(-1), out=gstart[1:])
        for w in range(NWIN):
            for h in range(2):
                coff = int(chunk_base[w][h])
                for bn in range(_nbins(w)):
                    t = int(T[w][h][bn])
                    n_real = int(cnt[w][h][bn])
                    assert n_real <= t * 128
                    g0 = int(gstart[(w * 2 + h) * NB + bn])
                    off = _boff(w, bn)
                    sg = s[g0:g0 + n_real]
                    dg = d[g0:g0 + n_real] - w * WIN
                    sc = sg // NP
                    soff = sg % NP
                    par = soff & 1
                    if h == 0:
                        brow = sc * APAIR + (soff >> 1)
                    else:
                        brow = sc * BPAIR + ((soff - AH) >> 1)
                    e0 = coff * 128
                    idxs[e0:e0 + n_real] = brow.astype(np.int16)
                    if n_real:
                        seg = (dg - off).astype(F32)
                        assert seg.min() >= 0 and seg.max() < INDW
                        sl = slice(e0, e0 + n_real)
                        segsA[sl] = np.where(par == 0, seg, -1.0)
                        segsB[sl] = np.where(par == 1, seg, -1.0)
                    coff += t
        eidx = np.tile(idxs.reshape(-1, 16).T, (8, 1))          # [128, ntot/16]
        esegA = np.ascontiguousarray(segsA.reshape(nch, 128).T).astype(BF16)
        esegB = np.ascontiguousarray(segsB.reshape(nch, 128).T).astype(BF16)
        per_core.append(dict(eidx=eidx, esegA=esegA, esegB=esegB,
                             dinv=dinv[i * NP:(i + 1) * NP]))
    meta = dict(T=T, nch=nch, ntot=ntot, chunk_base=chunk_base,
                chunk_off=chunk_off, call_edges=call_edges, call_e0=call_e0)
    return meta, per_core


def fold_weights(conv_w):
    betas = np.log(THETA / np.arange(1, L + 1, dtype=F32) + 1.0)
    wp = np.zeros((128, L * H), F32)
    eye = np.eye(H, dtype=F32)
    for l in range(L):
        wl = (1.0 - betas[l]) * eye + betas[l] * conv_w[l]
        wp[:H, l * H:(l + 1) * H] = wl
        wp[H:, l * H:(l + 1) * H] = wl
    return wp


def build_inputs(meta, per_core, inputs):
    """Per-core in_maps for the device program."""
    x_param = np.asarray(inputs["x_param"], F32)
    lin0_w = np.asarray(inputs["lin0_w"], F32)
    lin0_b = np.asarray(inputs["lin0_b"], F32)
    conv_w = np.asarray(inputs["conv_w"], F32)
    wp = fold_weights(conv_w)
    mlp_w0 = np.asarray(inputs["mlp_w0"], F32)
    mlp_w1 = np.asarray(inputs["mlp_w1"], F32)
    mlp_w2 = np.asarray(inputs["mlp_w2"], F32)
    out_w = np.asarray(inputs["out_w"], F32)

    iota96 = np.tile(np.arange(INDW, dtype=F32), (128, 1)).astype(BF16)
    iden2 = np.tile(np.eye(H, dtype=F32), (2, 1))               # [128, 64]
    lwt = np.zeros((125, 4, H), F32)
    for k in range(4):
        lwt[:, k, :] = lin0_w[k * 125:(k + 1) * 125, :]
    lb = np.zeros((H, 2), F32)
    lb[:, 0] = lin0_b
    lb[:, 1] = 0.1 * lin0_b
    w0d = np.tile(mlp_w0, (2, 1))                               # [128, 213]
    mw1a = mlp_w1[:128, :]
    mw1b = np.zeros((128, M2), F32)
    mw1b[:M1 - 128, :] = mlp_w1[128:, :]
    mw2 = np.zeros((3, 128, R), F32)
    mw2[0] = mlp_w2[:128]
    mw2[1] = mlp_w2[128:256]
    mw2[2, :M2 - 256] = mlp_w2[256:]
    owt = np.zeros((128, 4, C), F32)
    for mchunk in range(4):
        owt[:, mchunk, :] = out_w[mchunk * 128:(mchunk + 1) * 128, :]
    mb0 = np.zeros((128, 2), F32)
    mb0[:, 0] = np.asarray(inputs["mlp_b0"], F32)[:128]
    mb0[:M1 - 128, 1] = np.asarray(inputs["mlp_b0"], F32)[128:]
    mb1 = np.zeros((128, 3), F32)
    mb1[:, 0] = np.asarray(inputs["mlp_b1"], F32)[:128]
    mb1[:, 1] = np.asarray(inputs["mlp_b1"], F32)[128:256]
    mb1[:M2 - 256, 2] = np.asarray(inputs["mlp_b1"], F32)[256:]
    mb2 = np.zeros((128, 4), F32)
    for mchunk in range(4):
        mb2[:, mchunk] = np.asarray(inputs["mlp_b2"], F32)[mchunk * 128:(mchunk + 1) * 128]
    ob = np.tile(np.asarray(inputs["out_b"], F32), (128, 1))    # [128, 40]

    in_maps = []
    for i in range(NCORES):
        pc = per_core[i]
        dn = np.ones((128, 98), F32)
        dv = pc["dinv"]
        full = (NP // 128) * 128
        dn[:, :NP // 128] = dv[:full].reshape(-1, 128).T
        dn[:NP - full, NP // 128] = dv[full:]
        in_maps.append({
            "eidx": pc["eidx"], "esegA": pc["esegA"], "esegB": pc["esegB"],
            "xp": x_param[i * NP:(i + 1) * NP],
            "dinv_nm": dn, "iota96": iota96, "iden2": iden2,
            "wp": wp, "lwt": lwt, "lb": lb,
            "w0d": w0d, "mw1a": mw1a, "mw1b": mw1b,
            "mw2a": mw2[0], "mw2b": mw2[1], "mw2c": mw2[2],
            "owt": owt, "mb0": mb0, "mb1": mb1, "mb2": mb2, "ob": ob,
        })
    return in_maps


def build_program(meta, nlayers=NLAYERS):
    import concourse.bass as bass
    import concourse.bacc as bacc
    import concourse.mybir as mybir
    import concourse.tile as tile
    from concourse.bass import ds
    from concourse.masks import make_identity

    T = meta["T"]
    nch = meta["nch"]
    ntot = meta["ntot"]
    chunk_base = meta["chunk_base"]
    chunk_off = meta["chunk_off"]
    call_edges = meta["call_edges"]
    call_e0 = meta["call_e0"]
    PE = mybir.EngineType.PE
    f32 = mybir.dt.float32
    bf16 = mybir.dt.bfloat16
    AF = mybir.ActivationFunctionType
    ALU = mybir.AluOpType

    nc = bacc.Bacc("TRN2", target_bir_lowering=False, debug=False,
                   num_devices=NCORES, num_swdge_queues=NQUEUES)
    # ---- I/O ----
    eidx_in = nc.declare_dram_parameter("eidx", [128, ntot // 16], mybir.dt.int16, isOutput=False)
    esegA_in = nc.declare_dram_parameter("esegA", [128, nch], bf16, isOutput=False)
    esegB_in = nc.declare_dram_parameter("esegB", [128, nch], bf16, isOutput=False)
    xp_in = nc.declare_dram_parameter("xp", [NP, F], f32, isOutput=False)
    dinv_in = nc.declare_dram_parameter("dinv_nm", [128, 98], f32, isOutput=False)
    iota_in = nc.declare_dram_parameter("iota96", [128, INDW], bf16, isOutput=False)
    iden2_in = nc.declare_dram_parameter("iden2", [128, H], f32, isOutput=False)
    wp_in = nc.declare_dram_parameter("wp", [128, L * H], f32, isOutput=False)
    lwt_in = nc.declare_dram_parameter("lwt", [125, 4 * H], f32, isOutput=False)
    lb_in = nc.declare_dram_parameter("lb", [H, 2], f32, isOutput=False)
    w0d_in = nc.declare_dram_parameter("w0d", [128, M1], f32, isOutput=False)
    mw1a_in = nc.declare_dram_parameter("mw1a", [128, M2], f32, isOutput=False)
    mw1b_in = nc.declare_dram_parameter("mw1b", [128, M2], f32, isOutput=False)
    mw2a_in = nc.declare_dram_parameter("mw2a", [128, R], f32, isOutput=False)
    mw2b_in = nc.declare_dram_parameter("mw2b", [128, R], f32, isOutput=False)
    mw2c_in = nc.declare_dram_parameter("mw2c", [128, R], f32, isOutput=False)
    owt_in = nc.declare_dram_parameter("owt", [128, 4 * C], f32, isOutput=False)
    mb0_in = nc.declare_dram_parameter("mb0", [128, 2], f32, isOutput=False)
    mb1_in = nc.declare_dram_parameter("mb1", [128, 3], f32, isOutput=False)
    mb2_in = nc.declare_dram_parameter("mb2", [128, 4], f32, isOutput=False)
    ob_in = nc.declare_dram_parameter("ob", [128, C], f32, isOutput=False)
    out_d = nc.declare_dram_parameter("out", [NP, C], f32, isOutput=True)
    DBG = bool(os.environ.get("GCN_DEBUG"))
    if DBG:
        dbgx_d = nc.declare_dram_parameter(
            "dbgx", [128, ((NWIN + 1) // 2) * WIN], f32, isOutput=True)
        dbgt1_d = nc.declare_dram_parameter("dbgt1", [128, NP], f32, isOutput=True)
        dbgt2_d = nc.declare_dram_parameter("dbgt2", [128, NP], bf16, isOutput=True)
        dbgxfa_d = nc.declare_dram_parameter("dbgxfa", [AROWS, 128], bf16, isOutput=True)
        dbgxfb_d = nc.declare_dram_parameter("dbgxfb", [BROWS, 128], bf16, isOutput=True)
        dbgab_d = nc.declare_dram_parameter("dbgab", [H, NP], f32, isOutput=True)
        dbgh_d = nc.declare_dram_parameter("dbgh", [H, NP], f32, isOutput=True)

    xfullA = [nc.dram_tensor(f"xfullA{p}", [AROWS, 128], bf16, addr_space="Shared")
              for p in range(2)]
    xfullB = [nc.dram_tensor(f"xfullB{p}", [BROWS, 128], bf16, addr_space="Shared")
              for p in range(2)]
    slabA = [nc.dram_tensor(f"slabA{p}", [AH, H], bf16) for p in range(2)]
    slabB = [nc.dram_tensor(f"slabB{p}", [BH, H], bf16) for p in range(2)]

    NJ = 98  # node-major 128-subchunks (last = 84 rows)

    def xfin_loc(w):
        return 64 * (w % 2), (w // 2) * WIN

    with tile.TileContext(nc) as tc:
        with (
            tc.tile_pool(name="cst", bufs=1) as cst,
        ):
            # ---- resident tiles ----
            eidx = cst.tile([128, ntot // 16], mybir.dt.int16)
            esegA = cst.tile([128, nch], bf16)
            esegB = cst.tile([128, nch], bf16)
            dinv_nm = cst.tile([128, 98], f32)
            iota96 = cst.tile([128, INDW], bf16)
            iden2 = cst.tile([128, H], f32)
            iden = cst.tile([128, 128], f32)
            wp = cst.tile([128, L * H], f32)
            # T1: rows 0:64 = 0.9*dinv (f32), rows 64:128 = aggA (per-layer)
            t1 = cst.tile([128, NP], f32)
            # T2: rows 0:64 = 0.1*x0 (bf16), rows 64:128 = 0.9*dinv^2 (bf16)
            t2 = cst.tile([128, NP], bf16)
            x_fin = cst.tile([128, ((NWIN + 1) // 2) * WIN], f32)
            lwt = cst.tile([125, 4 * H], f32)
            lb = cst.tile([H, 2], f32)
            ones09 = cst.tile([128, H], f32)

            nc.sync.dma_start(eidx[:], eidx_in[:])
            nc.sync.dma_start(esegA[:], esegA_in[:])
            nc.sync.dma_start(esegB[:], esegB_in[:])
            nc.sync.dma_start(dinv_nm[:], dinv_in[:])
            nc.sync.dma_start(iota96[:], iota_in[:])
            nc.sync.dma_start(iden2[:], iden2_in[:])
            nc.sync.dma_start(wp[:], wp_in[:])
            nc.sync.dma_start(lwt[:], lwt_in[:])
            nc.sync.dma_start(lb[:], lb_in[:])
            make_identity(nc, iden[:])
            nc.gpsimd.memset(ones09[:], 0.9)

            def tail_window(w, stpool, out_par, psC):
                """x_fin window -> transpose -> dinv scale -> bf16 pair slab."""
                xb, xc = xfin_loc(w)
                winw = _winw(w)
                stage = stpool.tile([128, 4 * H], bf16, tag="stage")
                ng = (winw + 127) // 128
                for g in range(ng):
                    gn = min(128, winw - g * 128)
                    pst = psC.tile([128, H], f32, space="PSUM", tag="psC")
                    nc.tensor.transpose(
                        pst[:gn, :],
                        x_fin[xb:xb + H, xc + g * 128: xc + g * 128 + gn],
                        iden2[xb:xb + H, :],
                    )
                    nc.scalar.activation(
                        stage[:gn, g * H:(g + 1) * H], pst[:gn, :], AF.Copy,
                        scale=dinv_nm[:gn, (4 * w + g):(4 * w + g) + 1],
                    )
                if w < 12:
                    slab, base = slabA[out_par], w * WIN
                else:
                    slab, base = slabB[out_par], (w - 12) * WIN
                if winw == WIN:
                    nc.sync.dma_start(
                        slab[base:base + WIN, :].rearrange(
                            "(g p) f -> p g f", p=128),
                        stage[:].rearrange("p (g f) -> p g f", f=H),
                    )
                else:
                    for g in range(ng):
                        gn = min(128, winw - g * 128)
                        nc.sync.dma_start(
                            slab[base + g * 128:base + g * 128 + gn, :],
                            stage[:gn, g * H:(g + 1) * H],
                        )

            def allgather(which, out_par):
                slab = (slabA if which == 0 else slabB)[out_par]
                xf = (xfullA if which == 0 else xfullB)[out_par]
                if os.environ.get("GCN_NO_CC"):
                    npair = APAIR if which == 0 else BPAIR
                    nc.sync.dma_start(
                        xf[:npair, :],
                        slab[:].rearrange("(q h) f -> q (h f)", h=2))
                    return
                nc.gpsimd.collective_compute(
                    "AllGather", ALU.bypass,
                    replica_groups=[list(range(NCORES))],
                    ins=[slab[:]],
                    outs=[xf[:]],
                )

            # ================= init: T1/T2, x0, first slabs =================
            with (
                tc.tile_pool(name="initp", bufs=2) as initp,
                tc.tile_pool(name="initw", bufs=3) as initw,
                tc.tile_pool(name="ipsB", bufs=2, space="PSUM") as psB,
                tc.tile_pool(name="ipsC", bufs=2, space="PSUM") as psC,
            ):
                # wd = 0.9*dinv  (T1 rows 0:64); wd2 = 0.9*dinv^2 (T2 rows 64:)
                for j in range(NJ):
                    gn = min(128, NP - j * 128)
                    diag = initw.tile([128, 128], f32, tag="diag")
                    nc.vector.tensor_scalar_mul(
                        diag[:], iden[:], dinv_nm[:, j:j + 1])
                    diag2 = initw.tile([128, 128], f32, tag="diag2")
                    nc.vector.tensor_scalar_mul(
                        diag2[:], diag[:], dinv_nm[:, j:j + 1])
                    pw = psB.tile([H, 128], f32, space="PSUM", tag="psB")
                    nc.tensor.matmul(pw[:, :gn], lhsT=ones09[:],
                                     rhs=diag[:, :gn], start=True, stop=True)
                    nc.scalar.activation(
                        t1[:H, j * 128:j * 128 + gn], pw[:, :gn], AF.Copy)
                    pw2 = psB.tile([H, 128], f32, space="PSUM", tag="psB")
                    nc.tensor.matmul(pw2[:, :gn], lhsT=ones09[:],
                                     rhs=diag2[:, :gn], start=True, stop=True)
                    nc.scalar.activation(
                        t2[H:, j * 128:j * 128 + gn], pw2[:, :gn], AF.Copy)
                # x0 = relu(xp @ lin0_w + b)
                for c in range(NWIN):
                    winw = _winw(c)
                    ng = (winw + 127) // 128
                    xpt = initp.tile([128, 4 * F], f32, tag="xpt")
                    base = c * WIN
                    if winw == WIN:
                        nc.sync.dma_start(
                            xpt[:].rearrange("p (g f) -> p g f", f=F),
                            xp_in[base:base + WIN, :].rearrange(
                                "(g p) f -> p g f", p=128),
                        )
                    else:
                        for g in range(ng):
                            gn = min(128, winw - g * 128)
                            nc.sync.dma_start(
                                xpt[:gn, g * F:(g + 1) * F],
                                xp_in[base + g * 128:base + g * 128 + gn, :],
                            )
                    px0 = psB.tile([H, WIN], f32, space="PSUM", tag="psB")
                    for k in range(4):
                        xt = initw.tile([125, WIN], f32, tag=f"xt{k}")
                        for g in range(ng):
                            gn = min(128, winw - g * 128)
                            ptr = psC.tile([128, 128], f32, space="PSUM",
                                           tag="psC")
                            nc.tensor.transpose(
                                ptr[:125, :gn],
                                xpt[:gn, g * F + 125 * k: g * F + 125 * (k + 1)],
                                iden[:gn, :gn],
                            )
                            nc.scalar.activation(
                                xt[:, g * 128:g * 128 + gn], ptr[:125, :gn],
                                AF.Copy)
                        nc.tensor.matmul(
                            px0[:, :winw], lhsT=lwt[:, k * H:(k + 1) * H],
                            rhs=xt[:, :winw], start=(k == 0), stop=(k == 3))
                    xb, xc = xfin_loc(c)
                    nc.scalar.activation(
                        x_fin[xb:xb + H, xc:xc + winw], px0[:, :winw],
                        AF.Relu, bias=lb[:, 0:1])
                    nc.scalar.activation(
                        t2[:H, base:base + winw], px0[:, :winw],
                        AF.Relu, bias=lb[:, 1:2], scale=0.1)
                for w in range(NWIN):
                    tail_window(w, initw, 0, psC)
                allgather(0, 0)
                allgather(1, 0)

            # ================= layers =================
            SAFE = bool(os.environ.get("GCN_SAFE"))
            GPBUFS, IPBUFS, PSABUFS = (1, 1, 2) if SAFE else (4, 2, 4)
            with (
                tc.tile_pool(name="gp", bufs=GPBUFS) as gpool,
                tc.tile_pool(name="ip", bufs=IPBUFS) as ipool,
                tc.tile_pool(name="hp", bufs=2) as hpool,
                tc.tile_pool(name="stp", bufs=2) as stpool,
                tc.tile_pool(name="psA", bufs=PSABUFS, space="PSUM") as psA,
                tc.tile_pool(name="psB", bufs=2, space="PSUM") as psB,
                tc.tile_pool(name="psC", bufs=2, space="PSUM") as psC,
            ):
                cbmax = int(call_edges.max()) // 128
                tmax = max(len(chunk_off[w][ph])
                           for w in range(NWIN) for ph in range(2))
                DEP_SYNC = SAFE
                ncall = [0]
                g_hist = []          # last matmul consuming each gather tile
                ind_hist = {0: [], 1: []}  # per-parity indicator histories
                psw_hist = []        # op that releases each psA tile
                for lay in range(nlayers):
                    par = lay % 2

                    def do_block(ph, b, xf):
                        """Gathers + indicator matmuls for block b, phase ph.
                        Returns {w: psw tile}."""
                        ce = int(call_edges[ph][b])
                        wlist = list(range(b * WBLOCK,
                                           min((b + 1) * WBLOCK, NWIN)))
                        pswt = {}
                        if ce == 0:
                            return pswt
                        e0 = int(call_e0[ph][b])
                        g = gpool.tile([128, cbmax * 128], bf16, tag="gath")
                        gref = nc.gpsimd.dma_gather(
                            out_ap=g[:, :ce].rearrange(
                                "p (c d) -> p c d", d=128),
                            in_ap=xf[:, :],
                            idxs_ap=eidx[:, e0 // 16:(e0 + ce) // 16],
                            num_idxs=ce,
                            num_idxs_reg=ce,
                            elem_size=128,
                            single_packet=False,
                            queue_num=ncall[0] % NQUEUES,
                        )
                        ncall[0] += 1
                        if len(g_hist) >= GPBUFS and g_hist[-GPBUFS] is not None:
                            tile.add_dep_helper(gref.ins, g_hist[-GPBUFS].ins,
                                                sync=DEP_SYNC)
                        g_hist.append(None)
                        gslot = len(g_hist) - 1
                        gchunk0 = e0 // 128
                        last_mm = None
                        eseg = (esegA, esegB)
                        for w in wlist:
                            offs_w = chunk_off[w][ph]
                            t = len(offs_w)
                            if t == 0:
                                continue
                            psw = psA.tile([H, WIN], f32, space="PSUM",
                                           tag="psw", name=f"psw{ph}_{w}")
                            pswt[w] = psw
                            c0 = int(chunk_base[w][ph])
                            jj0 = c0 - gchunk0
                            inds = []
                            for p in range(2):
                                ind = ipool.tile([128, tmax * INDW], bf16,
                                                 tag=f"ind{p}")
                                ie = nc.vector.tensor_tensor(
                                    out=ind[:, :t * INDW].rearrange(
                                        "p (c s) -> p c s", s=INDW),
                                    in0=iota96[:].rearrange(
                                        "p (u s) -> p u s", u=1
                                    ).to_broadcast([128, t, INDW]),
                                    in1=eseg[p][:, c0:c0 + t].rearrange(
                                        "p (c u) -> p c u", u=1
                                    ).to_broadcast([128, t, INDW]),
                                    op=ALU.is_equal,
                                )
                                hist = ind_hist[p]
                                if len(hist) >= IPBUFS and hist[-IPBUFS] is not None:
                                    tile.add_dep_helper(
                                        ie.ins, hist[-IPBUFS].ins, sync=DEP_SYNC)
                                inds.append(ind)
                            nmm = 2 * t
                            k = 0
                            for j in range(t):
                                off = int(offs_w[j])
                                for p in range(2):
                                    mm = nc.tensor.matmul(
                                        psw[:, off:off + INDW],
                                        lhsT=g[:, (jj0 + j) * 128 + p * H:
                                               (jj0 + j) * 128 + (p + 1) * H],
                                        rhs=inds[p][:, j * INDW:(j + 1) * INDW],
                                        start=(k == 0),
                                        stop=(k == nmm - 1),
                                    )
                                    if k == 0 and len(psw_hist) >= PSABUFS:
                                        tile.add_dep_helper(
                                            mm.ins, psw_hist[-PSABUFS].ins,
                                            sync=DEP_SYNC)
                                    k += 1
                                    last_mm = mm
                            ind_hist[0].append(last_mm)
                            ind_hist[1].append(last_mm)
                        g_hist[gslot] = last_mm
                        return pswt

                    # ---------- phase A: bucket-A chunks -> aggA ----------
                    for b in range(NBLK):
                        pswt = do_block(0, b, xfullA[par])
                        for w, psw in pswt.items():
                            winw = _winw(w)
                            cp = nc.scalar.activation(
                                t1[H:, w * WIN:w * WIN + winw],
                                psw[:, :winw], AF.Copy)
                            psw_hist.append(cp)
                    # ---------- phase B ----------
                    for b in range(NBLK):
                        pswt = do_block(1, b, xfullB[par])
                        for w in range(b * WBLOCK,
                                       min((b + 1) * WBLOCK, NWIN)):
                            winw = _winw(w)
                            xb, xc = xfin_loc(w)
                            wc = slice(w * WIN, w * WIN + winw)
                            h1 = hpool.tile([H, WIN], f32, tag="h1")
                            h2 = hpool.tile([H, WIN], f32, tag="h2")
                            psw = pswt[w]
                            if DBG and lay == nlayers - 1:
                                dst = hpool.tile([H, WIN], f32, tag="dbgst")
                                nc.scalar.activation(
                                    dst[:, :winw], psw[:, :winw], AF.Copy)
                                nc.sync.dma_start(
                                    dbgab_d[:, w * WIN:w * WIN + winw],
                                    dst[:, :winw])
                            s1 = nc.vector.tensor_tensor(
                                h1[:, :winw], psw[:, :winw],
                                t1[H:, wc], op=ALU.add)
                            psw_hist.append(s1)
                            nc.vector.tensor_tensor(
                                h2[:, :winw], h1[:, :winw],
                                t1[:H, wc], op=ALU.mult)
                            # + 0.1*x0
                            nc.vector.tensor_tensor(
                                h1[:, :winw], h2[:, :winw],
                                t2[:H, wc], op=ALU.add)
                            # + 0.9*dinv^2 * x_old (self loop); the DVE needs
                            # both SB inputs on the same base partition, so
                            # even windows stage x_old through partitions 64+
                            if xb == 0:
                                xold = hpool.tile([128, WIN], f32, tag="xold")
                                nc.scalar.activation(
                                    xold[64:, :winw],
                                    x_fin[0:H, xc:xc + winw], AF.Copy)
                                xold_ap = xold[64:, :winw]
                            else:
                                xold_ap = x_fin[xb:xb + H, xc:xc + winw]
                            nc.vector.tensor_tensor(
                                h2[:, :winw], xold_ap,
                                t2[H:, wc], op=ALU.mult)
                            nc.vector.tensor_tensor(
                                h1[:, :winw], h1[:, :winw],
                                h2[:, :winw], op=ALU.add)
                            if DBG and lay == nlayers - 1:
                                dsh = hpool.tile([H, WIN], f32, tag="dbgst")
                                nc.scalar.activation(
                                    dsh[:, :winw], h1[:, :winw], AF.Copy)
                                nc.sync.dma_start(
                                    dbgh_d[:, w * WIN:w * WIN + winw],
                                    dsh[:, :winw])
                            ps2 = psB.tile([H, WIN], f32, space="PSUM",
                                           tag="psB")
                            nc.tensor.matmul(
                                ps2[:, :winw],
                                lhsT=wp[:H, lay * H:(lay + 1) * H],
                                rhs=h1[:, :winw], start=True, stop=True)
                            nc.scalar.activation(
                                x_fin[xb:xb + H, xc:xc + winw],
                                ps2[:, :winw], AF.Relu)
                            if lay < nlayers - 1:
                                tail_window(w, stpool, 1 - par, psC)
                        if lay < nlayers - 1 and b == AGA_BLK:
                            allgather(0, 1 - par)
                    if lay < nlayers - 1:
                        allgather(1, 1 - par)

            if DBG:
                nc.sync.dma_start(dbgx_d[:], x_fin[:])
                nc.sync.dma_start(dbgt1_d[:], t1[:])
                nc.sync.dma_start(dbgt2_d[:], t2[:])
                nc.sync.dma_start(dbgxfa_d[:], xfullA[0][:])
                nc.sync.dma_start(dbgxfb_d[:], xfullB[0][:])

            # ================= MLP head =================
            with (
                tc.tile_pool(name="mh", bufs=2) as mh,
                tc.tile_pool(name="mw", bufs=1) as mw,
                tc.tile_pool(name="mpsA", bufs=4, space="PSUM") as psA,
                tc.tile_pool(name="mpsB", bufs=2, space="PSUM") as psB,
                tc.tile_pool(name="mpsC", bufs=2, space="PSUM") as psC,
            ):
                w0d = mw.tile([128, M1], f32)
                mw1a = mw.tile([128, M2], f32)
                mw1b = mw.tile([128, M2], f32)
                mw2a = mw.tile([128, R], f32)
                mw2b = mw.tile([128, R], f32)
                mw2c = mw.tile([128, R], f32)
                owt = mw.tile([128, 4 * C], f32)
                mb0 = mw.tile([128, 2], f32)
                mb1 = mw.tile([128, 3], f32)
                mb2 = mw.tile([128, 4], f32)
                ob = mw.tile([128, C], f32)
                nc.sync.dma_start(w0d[:], w0d_in[:])
                nc.sync.dma_start(mw1a[:], mw1a_in[:])
                nc.sync.dma_start(mw1b[:], mw1b_in[:])
                nc.sync.dma_start(mw2a[:], mw2a_in[:])
                nc.sync.dma_start(mw2b[:], mw2b_in[:])
                nc.sync.dma_start(mw2c[:], mw2c_in[:])
                nc.sync.dma_start(owt[:], owt_in[:])
                nc.sync.dma_start(mb0[:], mb0_in[:])
                nc.sync.dma_start(mb1[:], mb1_in[:])
                nc.sync.dma_start(mb2[:], mb2_in[:])
                nc.sync.dma_start(ob[:], ob_in[:])
                for c in range(NWIN):
                    winw = _winw(c)
                    xb, xc = xfin_loc(c)
                    xin = x_fin[xb:xb + H, xc:xc + winw]
                    # h1 = relu(x @ w0 + b0): [213, winw]
                    h1a = mh.tile([128, WIN], f32, tag="h1a")
                    h1b = mh.tile([M1 - 128, WIN], f32, tag="h1b")
                    p1 = psB.tile([128, WIN], f32, space="PSUM", tag="psB")
                    nc.tensor.matmul(p1[:, :winw], lhsT=w0d[xb:xb + H, :128],
                                     rhs=xin, start=True, stop=True)
                    nc.scalar.activation(h1a[:, :winw], p1[:, :winw], AF.Relu,
                                         bias=mb0[:, 0:1])
                    p1b = psB.tile([M1 - 128, WIN], f32, space="PSUM",
                                   tag="psB")
                    nc.tensor.matmul(p1b[:, :winw],
                                     lhsT=w0d[xb:xb + H, 128:M1],
                                     rhs=xin, start=True, stop=True)
                    nc.scalar.activation(h1b[:, :winw], p1b[:, :winw], AF.Relu,
                                         bias=mb0[:M1 - 128, 1:2])
                    # h2 = relu(h1 @ w1 + b1): [362, winw]
                    h2 = []
                    for mchunk in range(3):
                        mn = min(128, M2 - mchunk * 128)
                        p2 = psA.tile([128, WIN], f32, space="PSUM", tag="psw")
                        nc.tensor.matmul(
                            p2[:mn, :winw],
                            lhsT=mw1a[:, mchunk * 128:mchunk * 128 + mn],
                            rhs=h1a[:, :winw], start=True, stop=False)
                        nc.tensor.matmul(
                            p2[:mn, :winw],
                            lhsT=mw1b[:M1 - 128, mchunk * 128:mchunk * 128 + mn],
                            rhs=h1b[:M1 - 128, :winw], start=False, stop=True)
                        h2t = mh.tile([128, WIN], f32, tag=f"h2_{mchunk}")
                        nc.scalar.activation(h2t[:mn, :winw], p2[:mn, :winw],
                                             AF.Relu,
                                             bias=mb1[:mn, mchunk:mchunk + 1])
                        h2.append(h2t)
                    # h3 = h2 @ w2 + b2 (no relu): [512, winw]
                    h3 = []
                    w2t = [mw2a, mw2b, mw2c]
                    kn = [128, 128, M2 - 256]
                    for mchunk in range(4):
                        p3 = psA.tile([128, WIN], f32, space="PSUM", tag="psw")
                        for k in range(3):
                            nc.tensor.matmul(
                                p3[:, :winw],
                                lhsT=w2t[k][:kn[k],
                                            mchunk * 128:(mchunk + 1) * 128],
                                rhs=h2[k][:kn[k], :winw],
                                start=(k == 0), stop=(k == 2))
                        h3t = mh.tile([128, WIN], f32, tag=f"h3_{mchunk}")
                        nc.scalar.activation(h3t[:, :winw], p3[:, :winw],
                                             AF.Identity,
                                             bias=mb2[:, mchunk:mchunk + 1])
                        h3.append(h3t)
                    # out = h3 @ out_w + out_b: node-major [winw, 40]
                    ost = mh.tile([128, 4 * C], f32, tag="ost")
                    ng = (winw + 127) // 128
                    for g in range(ng):
                        gn = min(128, winw - g * 128)
                        po = psC.tile([128, C], f32, space="PSUM", tag="psC")
                        for k in range(4):
                            nc.tensor.matmul(
                                po[:gn, :],
                                lhsT=h3[k][:, g * 128:g * 128 + gn],
                                rhs=owt[:, k * C:(k + 1) * C],
                                start=(k == 0), stop=(k == 3))
                        nc.vector.tensor_tensor(
                            ost[:gn, g * C:(g + 1) * C], po[:gn, :],
                            ob[:gn, :], op=ALU.add)
                    base = c * WIN
                    if winw == WIN:
                        nc.sync.dma_start(
                            out_d[base:base + WIN, :].rearrange(
                                "(g p) f -> p g f", p=128),
                            ost[:].rearrange("p (g f) -> p g f", f=C),
                        )
                    else:
                        for g in range(ng):
                            gn = min(128, winw - g * 128)
                            nc.sync.dma_start(
                                out_d[base + g * 128:base + g * 128 + gn, :],
                                ost[:gn, g * C:(g + 1) * C],
                            )
    nc.finalize()
    return nc


_CACHE = {}


def _get_program(meta, nlayers):
    key = (meta["nch"], nlayers, meta["T"].tobytes(),
           bool(os.environ.get("GCN_DEBUG")), bool(os.environ.get("GCN_SAFE")))
    if key not in _CACHE:
        _CACHE[key] = build_program(meta, nlayers)
    return _CACHE[key]


_LAST_EXEC_NS = None


def kernel(**inputs):
    global _LAST_EXEC_NS
    from concourse.bass_utils import run_bass_kernel_spmd
    edge_index = np.asarray(inputs["edge_index"])
    meta, per_core = preprocess(edge_index)
    in_maps = build_inputs(meta, per_core, inputs)
    nc = _get_program(meta, NLAYERS)
    kw = {}
    if os.environ.get("GCN_TRACE"):
        kw["trace"] = True
        if os.environ.get("GCN_TRACE_DIR"):
            kw["tmpdir"] = os.environ["GCN_TRACE_DIR"]
    res = run_bass_kernel_spmd(nc, in_maps, list(range(NCORES)), **kw)
    if getattr(res, "exec_time_ns", None):
        _LAST_EXEC_NS = res.exec_time_ns
    globals()["_LAST_RESULTS"] = res.results
    out = np.concatenate([res.results[i]["out"] for i in range(NCORES)], axis=0)
    return out

